# revision 1
# baseline (speedup 1.0000x reference)
"""Trainium2 Bass kernel for nn_CostVolume (SpatialCorrelationSampler-style).

out[b, dy*9+dx, y, x] = sum_c feat1[b,c,y,x] * feat2_pad[b,c,y+dy,x+dx]
with feat2 zero-padded by 4 on H/W, dy/dx in [0,9), B=4, C=256, H=W=96.

Sharding (8 cores): core = (b, half) — batch x H-half. Per core:
  f1  [256, 48, 96]    feat1[b, :, half*48:(half+1)*48, :]
  f2  [256, 56, 104]   pad(feat2[b])[:, half*48 : half*48+56, :]
  out O'[6, 3, 96, 8, 27] intermediate (deskewed on host)

Device algorithm per (y, g) (g = dy-group of 3):
  Gram G[x, n] = sum_c f1[c,y,x] * f2[c, y+3g+dyl, x'],  n = x'*3 + dyl
  computed as 2 accumulating f32r matmuls (C split 128+128) -> PSUM [96, 312].
  Useful band for partition x = 27 contiguous cols starting at 3x
  (j = 3*dx + dyl). PSUM -> SBUF staging (DVE/ACT), then one diagonal-AP
  DMA per (8-row block, g): src ap [[7488+3, 96], [936, 8], [1, 27]].
"""

import numpy as np
import ml_dtypes

import concourse.bacc as bacc
import concourse.mybir as mybir
from concourse.ap import AP
from concourse.tile import TileContext
from concourse.bass_utils import run_bass_kernel_spmd

B, C, H, W = 4, 256, 96, 96
D = 4            # max displacement; pad width
P = 2 * D + 1    # 9 displacements per axis
HH = H // 2      # 48 rows per core
IB = 16          # input y-block
SB = 8           # staging y-block
NIB = HH // IB   # 3 input blocks
NSB = IB // SB   # 2 staging sub-blocks per input block
WROW = 936 * SB  # staging row length (elements per partition)

F32 = mybir.dt.float32
F32R = mybir.dt.float32r
BF16 = mybir.dt.bfloat16
USE_BF16 = True
IN_DT = BF16 if USE_BF16 else F32R
STA_W = W   # stationary cols (FWL unavailable: ldw-opt incompatible)
MP = 96     # matmul output partitions

_CACHED = {}


def _build_nc():
    nc = bacc.Bacc()
    f1 = nc.declare_dram_parameter(
        "f1", [C, HH * W + (STA_W - W)], IN_DT, isOutput=False
    )
    f2 = nc.declare_dram_parameter("f2", [C, HH + 8, W + 8], IN_DT, isOutput=False)
    out = nc.declare_dram_parameter(
        "o", [HH // SB, 3, W, SB, 27], F32, isOutput=True
    )

    with TileContext(nc) as tc:
        with (
            tc.tile_pool(name="inp", bufs=2) as inp,
            tc.tile_pool(name="stage", bufs=3) as stp,
            tc.tile_pool(name="ps", bufs=2, space="PSUM") as psp,
        ):
            for blk in range(NIB):
                f1t = []
                f2t = []
                for ch in range(2):
                    t1 = inp.tile([128, IB * W + (STA_W - W)], IN_DT, tag=f"f1c{ch}")
                    nc.scalar.dma_start(
                        out=t1[:, :],
                        in_=f1[
                            ch * 128 : (ch + 1) * 128,
                            blk * IB * W : (blk + 1) * IB * W + (STA_W - W),
                        ],
                    )
                    f1t.append(t1)
                    t2 = inp.tile([128, IB + 8, W + 8], IN_DT, tag=f"f2c{ch}")
                    nc.scalar.dma_start(
                        out=t2[:, :, :],
                        in_=f2[
                            ch * 128 : (ch + 1) * 128,
                            blk * IB : blk * IB + IB + 8,
                            :,
                        ],
                    )
                    f2t.append(t2)

                for sub in range(NSB):
                    st = stp.tile([96, WROW], F32, tag="st")
                    for yl in range(SB):
                        yi = sub * SB + yl  # y within input block
                        # one 3-bank PSUM tile per y; matmul g at col g*512
                        ps = psp.tile([MP, 1536], F32, tag="ps")
                        for ch in range(2):
                            sta = f1t[ch][:, yi * W : yi * W + STA_W]
                            for g in range(3):
                                mov = f2t[ch][
                                    :, yi + 3 * g : yi + 3 * g + 3, :
                                ].rearrange("c r x -> c (r x)")
                                nc.tensor.matmul(
                                    ps[:, g * 512 : g * 512 + 312],
                                    lhsT=sta,
                                    rhs=mov,
                                    start=(ch == 0),
                                    stop=(ch == 1),
                                )
                        # single strided copy per y: PSUM (g, dyl, x') ->
                        # staging interleaved col = g*312 + x'*3 + dyl
                        psap = ps[0:96, :]
                        src = AP(
                            tensor=psap.tensor,
                            offset=psap.offset,
                            ap=[[1536, 96], [512, 3], [104, 3], [1, 104]],
                        )
                        stap0 = st[:, :]
                        dst = AP(
                            tensor=stap0.tensor,
                            offset=stap0.offset + yl * 936,
                            ap=[[WROW, 96], [312, 3], [1, 3], [3, 104]],
                        )
                        if yl % 2 == 0:
                            nc.vector.tensor_copy(dst, src)
                        else:
                            nc.scalar.copy(out=dst, in_=src)
                    # band extraction: one diagonal-AP DMA per g, 27-elem runs
                    stap = st[:, :]
                    for g in range(3):
                        src = AP(
                            tensor=stap.tensor,
                            offset=stap.offset + g * 312,
                            ap=[[WROW + 3, 96], [936, SB], [1, 27]],
                        )
                        eng = nc.sync if (sub * 3 + g) % 2 == 0 else nc.scalar
                        eng.dma_start(out=out[blk * NSB + sub, g], in_=src)
    nc.finalize()
    return nc


def kernel(feat1: np.ndarray, feat2: np.ndarray) -> np.ndarray:
    feat1 = np.ascontiguousarray(np.asarray(feat1, dtype=np.float32))
    feat2 = np.ascontiguousarray(np.asarray(feat2, dtype=np.float32))

    if "nc" not in _CACHED:
        _CACHED["nc"] = _build_nc()
    nc = _CACHED["nc"]

    core_ids = list(range(8))
    in_maps = []
    for core in core_ids:
        b, half = divmod(core, 2)
        f1s = feat1[b][:, half * HH : (half + 1) * HH, :].reshape(C, HH * W)
        f1s = np.concatenate(
            [f1s, np.zeros((C, STA_W - W), np.float32)], axis=1
        )
        f2p = np.pad(feat2[b], ((0, 0), (D, D), (D, D)))
        f2s = f2p[:, half * HH : half * HH + HH + 8, :]
        npdt = ml_dtypes.bfloat16 if USE_BF16 else np.float32
        in_maps.append(
            {
                "f1": np.ascontiguousarray(f1s.astype(npdt)),
                "f2": np.ascontiguousarray(f2s.astype(npdt)),
            }
        )

    res = run_bass_kernel_spmd(nc, in_maps, core_ids)

    out = np.empty((B, P * P, H, W), np.float32)
    for core in core_ids:
        b, half = divmod(core, 2)
        Op = res.results[core]["o"]  # [cb, g, x, yl, 27] with j = 3*dx + dyl
        O = Op.reshape(HH // SB, 3, W, SB, P, 3)  # cb, g, x, yl, dx, dyl
        core_out = O.transpose(1, 5, 4, 0, 3, 2).reshape(P * P, HH, W)
        out[b, :, half * HH : (half + 1) * HH, :] = core_out
    return out



# revision 17
# speedup vs baseline: 1.9581x; 1.9581x over previous
"""Trainium2 Bass kernel for nn_CostVolume (SpatialCorrelationSampler-style).

out[b, dy*9+dx, y, x] = sum_c feat1[b,c,y,x] * feat2_pad[b,c,y+dy,x+dx]
with feat2 zero-padded by 4 on H/W, dy/dx in [0,9), B=4, C=256, H=W=96.

Sharding (8 cores): core = (b, half) -- batch x H-half (48 rows each).

Device algorithm: 2D-tiled gram blocks. Per (y-block of 16, x-tile of 8):
  stationary = f1 tile [128c, 128=(16y x 8x)]  (128-col => FWL enabled)
  moving     = f2 window [128c, 384=(24r x 16x')] in 3 8-row chunks
  psum[m=(yi,xx), n=r_rel*16+x_rel] accumulated over 2 C-halves x 3 chunks.
  PSUM -> per-block SBUF stage [128, 12*384]. Output: per (blk, yi) one
  rectangular DMA [[4608, 8], [384, 12], [1, 144]] at (partition 8*yi,
  col 16*yi) reads the union window covering all 81 displacements for
  the 8 xx pixels (k = xx + 16*dy + dx); host strips via as_strided.
"""

import numpy as np
import ml_dtypes

import concourse.bacc as bacc
import concourse.mybir as mybir
from concourse.ap import AP
from concourse.tile import TileContext
from concourse.bass_utils import run_bass_kernel_spmd

B, C, H, W = 4, 256, 96, 96
D = 4
P = 2 * D + 1     # 9 displacements per axis
HH = H // 2       # 48 rows per core
NB = 3            # y-blocks of TY
TY, TX = 16, 8
NXT = W // TX     # 12 x-tiles
XW = TX + 8       # 16-col f2 window per tile
NCH = 7           # f2 row-chunks of 8
SPAN = 8 * XW + P  # 137
RL = SPAN + TX - 1  # 144: per-yi-group read window (union over xx)
STW = NXT * 384     # staged cols per block: 4608

F32 = mybir.dt.float32
BF16 = mybir.dt.bfloat16
STAGE_BF16 = False
ST_DT = BF16 if STAGE_BF16 else F32

_CACHED = {}


def _build_nc():
    nc = bacc.Bacc()
    f1 = nc.declare_dram_parameter("f1", [128, 2, NB, NXT * 128], BF16, isOutput=False)
    f2 = nc.declare_dram_parameter("f2", [128, 2, NCH, 8 * 104], BF16, isOutput=False)
    out = nc.declare_dram_parameter("o", [NB, TY, TX, NXT, RL], ST_DT, isOutput=True)

    with TileContext(nc) as tc:
        with (
            tc.tile_pool(name="w", bufs=1) as wp,
            tc.tile_pool(name="m", bufs=1) as mp,
            tc.tile_pool(name="st", bufs=3) as stp,
            tc.tile_pool(name="ps", bufs=4, space="PSUM") as psp,
        ):
            f1t = [None] * NB
            f2t = [None] * NCH

            def load_f1(blk):
                t = wp.tile([128, 2, NXT * 128], BF16, tag=f"f1b{blk}",
                            name=f"f1b{blk}")
                nc.sync.dma_start(out=t[:, :, :], in_=f1[:, :, blk, :])
                f1t[blk] = t

            def load_f2(k):
                t = mp.tile([128, 2, 8 * 104], BF16, tag=f"f2k{k}",
                            name=f"f2k{k}")
                nc.sync.dma_start(out=t[:, :, :], in_=f2[:, :, k, :])
                f2t[k] = t

            # priority load order: what block 0 needs first
            load_f1(0)
            for k in (0, 1, 2):
                load_f2(k)
            load_f1(1)
            for k in (3, 4):
                load_f2(k)
            load_f1(2)
            for k in (5, 6):
                load_f2(k)

            out_engines = [nc.scalar, nc.sync, nc.gpsimd]
            ti = 0
            oi = 0
            for blk in range(NB):
                st = stp.tile([128, STW], ST_DT, tag="st", name="st")
                for xt in range(NXT):
                    ps = psp.tile([128, 512], F32, tag="ps", name="ps")
                    for jj in range(3):
                        mt = f2t[2 * blk + jj]
                        for ch in range(2):
                            wt = f1t[blk]
                            lhsT = AP(
                                tensor=wt.tensor,
                                offset=wt.offset + ch * (NXT * 128) + xt * 128,
                                ap=[[2 * NXT * 128, 128], [1, 128]],
                            )
                            rhs = AP(
                                tensor=mt.tensor,
                                offset=mt.offset + ch * 832 + 8 * xt,
                                ap=[[1664, 128], [104, 8], [1, XW]],
                            )
                            nc.tensor.matmul(
                                ps[:, jj * 128 : (jj + 1) * 128],
                                lhsT=lhsT,
                                rhs=rhs,
                                start=(ch == 0),
                                stop=(ch == 1),
                            )
                    dst = st[:, xt * 384 : (xt + 1) * 384]
                    if ti % 3 != 2:
                        nc.vector.tensor_copy(dst, ps[:, 0:384])
                    else:
                        nc.scalar.copy(out=dst, in_=ps[:, 0:384])
                    ti += 1
                for yi in range(TY):
                    src = AP(
                        tensor=st.tensor,
                        offset=st.offset + yi * 8 * STW + XW * yi,
                        ap=[[STW, 8], [384, NXT], [1, RL]],
                    )
                    out_engines[oi % 3].dma_start(out=out[blk, yi], in_=src)
                    oi += 1
    nc.finalize()
    return nc


def kernel(feat1: np.ndarray, feat2: np.ndarray) -> np.ndarray:
    feat1 = np.ascontiguousarray(np.asarray(feat1, dtype=np.float32))
    feat2 = np.ascontiguousarray(np.asarray(feat2, dtype=np.float32))

    if "nc" not in _CACHED:
        _CACHED["nc"] = _build_nc()
    nc = _CACHED["nc"]

    core_ids = list(range(8))
    in_maps = []
    for core in core_ids:
        b, half = divmod(core, 2)
        f1h = feat1[b][:, half * HH : half * HH + HH, :]  # [256, 48, 96]
        # [c, blk, yi, xt, xx] -> [cl, ch, blk, (xt yi xx)]
        f1td = (
            f1h.reshape(256, NB, TY, NXT, TX)
            .transpose(0, 1, 3, 2, 4)
            .reshape(2, 128, NB, NXT * 128)
            .transpose(1, 0, 2, 3)
        )
        f2p = np.pad(feat2[b], ((0, 0), (D, D), (D, D)))[
            :, half * HH : half * HH + HH + 8, :
        ]  # [256, 56, 104]
        f2td = f2p.reshape(2, 128, NCH, 8 * 104).transpose(1, 0, 2, 3)
        in_maps.append(
            {
                "f1": np.ascontiguousarray(f1td.astype(ml_dtypes.bfloat16)),
                "f2": np.ascontiguousarray(f2td.astype(ml_dtypes.bfloat16)),
            }
        )

    res = run_bass_kernel_spmd(nc, in_maps, core_ids)

    out = np.empty((B, P * P, H, W), np.float32)
    for core in core_ids:
        b, half = divmod(core, 2)
        o = res.results[core]["o"]  # [3, 16, 8, 12, 144]
        o = np.ascontiguousarray(o).astype(np.float32)
        o5 = o.reshape(NB, TY, TX, NXT, RL)
        tmp = np.empty((P * P, NB, TY, NXT, TX), np.float32)
        for xx in range(TX):
            a = o5[:, :, xx, :, xx:]  # [3, 16, 12, >=137] view
            sa = a.strides
            v = np.lib.stride_tricks.as_strided(
                a, shape=(NB, TY, NXT, P, P),
                strides=(sa[0], sa[1], sa[2], XW * sa[3], sa[3]),
            )
            # v[blk, yi, xt, dy, dx]
            tmp[:, :, :, :, xx] = v.transpose(3, 4, 0, 1, 2).reshape(
                P * P, NB, TY, NXT
            )
        core_out = tmp.reshape(P * P, HH, W)
        out[b, :, half * HH : half * HH + HH, :] = core_out
    return out


# revision 19
# speedup vs baseline: 2.1737x; 1.1101x over previous
"""Trainium2 Bass kernel for nn_CostVolume (SpatialCorrelationSampler-style).

out[b, dy*9+dx, y, x] = sum_c feat1[b,c,y,x] * feat2_pad[b,c,y+dy,x+dx]
with feat2 zero-padded by 4 on H/W, dy/dx in [0,9), B=4, C=256, H=W=96.

Sharding (8 cores): core = (b, half) -- batch x H-half (48 rows each).

Device algorithm: 2D-tiled gram blocks. Per (y-block of 16, x-tile of 8):
  stationary = f1 tile [128c, 128=(16y x 8x)]  (128-col => FWL enabled)
  moving     = f2 window [128c, 384=(24r x 16x')], 1-2 MMs per C-half
  (split only at f2 group-tile boundaries; groups A=rows0-23, B=24-39,
  C=40-55). psum[m=(yi,xx), n=r_rel*16+x_rel] accumulated over 2 C-halves.
  PSUM -> per-block SBUF stage [128, 12*384] bf16 (cast on copy).
  Output: per (blk, 32-partition group g) one rectangular DMA
  [[4608, 32], [384, 12], [1, 192]] at (partition 32g, col 64g): the
  union window covering all 81 displacements for those 4 yi groups
  (k = 16*(yi%4) + xx + 16*dy + dx); host strips via as_strided.
"""

import numpy as np
import ml_dtypes

import concourse.bacc as bacc
import concourse.mybir as mybir
from concourse.ap import AP
from concourse.tile import TileContext
from concourse.bass_utils import run_bass_kernel_spmd

B, C, H, W = 4, 256, 96, 96
D = 4
P = 2 * D + 1     # 9 displacements per axis
HH = H // 2       # 48 rows per core
NB = 3            # y-blocks of TY
TY, TX = 16, 8
NXT = W // TX     # 12 x-tiles
XW = TX + 8       # 16-col f2 window per tile
RL = 192          # per-32-partition-group read window (union over 4 yi)
STW = NXT * 384   # staged cols per block: 4608
GW = (2496, 1664, 1664)  # f2 group widths (rows*104): A=24r, B=16r, C=16r
GR0 = (0, 24, 40)        # first f2 row of each group

F32 = mybir.dt.float32
BF16 = mybir.dt.bfloat16

_CACHED = {}


def _build_nc():
    nc = bacc.Bacc()
    f1 = nc.declare_dram_parameter("f1", [128, 2, NB, NXT * 128], BF16, isOutput=False)
    f2a = nc.declare_dram_parameter("f2a", [128, 2, GW[0]], BF16, isOutput=False)
    f2b = nc.declare_dram_parameter("f2b", [128, 2, GW[1]], BF16, isOutput=False)
    f2c = nc.declare_dram_parameter("f2c", [128, 2, GW[2]], BF16, isOutput=False)
    out = nc.declare_dram_parameter("o", [NB, 4, 32, NXT, RL], BF16, isOutput=True)

    with TileContext(nc) as tc:
        with (
            tc.tile_pool(name="w", bufs=1) as wp,
            tc.tile_pool(name="m", bufs=1) as mp,
            tc.tile_pool(name="st", bufs=3) as stp,
            tc.tile_pool(name="ps", bufs=4, space="PSUM") as psp,
        ):
            f1t = [None] * NB
            f2t = [None] * 3

            def load_f1(blk):
                t = wp.tile([128, 2, NXT * 128], BF16, tag=f"f1b{blk}",
                            name=f"f1b{blk}")
                nc.sync.dma_start(out=t[:, :, :], in_=f1[:, :, blk, :])
                f1t[blk] = t

            def load_f2(g, dram):
                t = mp.tile([128, 2, GW[g]], BF16, tag=f"f2g{g}",
                            name=f"f2g{g}")
                nc.sync.dma_start(out=t[:, :, :], in_=dram[:, :, :])
                f2t[g] = t

            load_f1(0)
            load_f2(0, f2a)
            load_f1(1)
            load_f2(1, f2b)
            load_f1(2)
            load_f2(2, f2c)

            # per block: list of (group, row0_local_in_group, nrows, psum_col0)
            mm_plan = [
                [(0, 0, 24, 0)],
                [(0, 16, 8, 0), (1, 0, 16, 128)],
                [(1, 8, 8, 0), (2, 0, 16, 128)],
            ]

            out_engines = [nc.scalar, nc.gpsimd, nc.sync]
            ti = 0
            oi = 0
            for blk in range(NB):
                st = stp.tile([128, STW], BF16, tag="st", name="st")
                for xt in range(NXT):
                    ps = psp.tile([128, 512], F32, tag="ps", name="ps")
                    for g, r0, nr, col0 in mm_plan[blk]:
                        for ch in range(2):
                            wt = f1t[blk]
                            lhsT = AP(
                                tensor=wt.tensor,
                                offset=wt.offset + ch * (NXT * 128) + xt * 128,
                                ap=[[2 * NXT * 128, 128], [1, 128]],
                            )
                            mt = f2t[g]
                            rhs = AP(
                                tensor=mt.tensor,
                                offset=mt.offset + ch * GW[g] + r0 * 104 + 8 * xt,
                                ap=[[2 * GW[g], 128], [104, nr], [1, XW]],
                            )
                            nc.tensor.matmul(
                                ps[:, col0 : col0 + nr * XW],
                                lhsT=lhsT,
                                rhs=rhs,
                                start=(ch == 0),
                                stop=(ch == 1),
                            )
                    dst = st[:, xt * 384 : (xt + 1) * 384]
                    if ti % 3 != 2:
                        nc.vector.tensor_copy(dst, ps[:, 0:384])
                    else:
                        nc.scalar.copy(out=dst, in_=ps[:, 0:384])
                    ti += 1
                for g in range(4):
                    src = AP(
                        tensor=st.tensor,
                        offset=st.offset + 32 * g * STW + 64 * g,
                        ap=[[STW, 32], [384, NXT], [1, RL]],
                    )
                    out_engines[oi % 3].dma_start(out=out[blk, g], in_=src)
                    oi += 1
    nc.finalize()
    return nc


def kernel(feat1: np.ndarray, feat2: np.ndarray) -> np.ndarray:
    feat1 = np.ascontiguousarray(np.asarray(feat1, dtype=np.float32))
    feat2 = np.ascontiguousarray(np.asarray(feat2, dtype=np.float32))

    if "nc" not in _CACHED:
        _CACHED["nc"] = _build_nc()
    nc = _CACHED["nc"]

    core_ids = list(range(8))
    in_maps = []
    for core in core_ids:
        b, half = divmod(core, 2)
        f1h = feat1[b][:, half * HH : half * HH + HH, :]  # [256, 48, 96]
        # [c, blk, yi, xt, xx] -> [cl, ch, blk, (xt yi xx)]
        f1td = (
            f1h.reshape(256, NB, TY, NXT, TX)
            .transpose(0, 1, 3, 2, 4)
            .reshape(2, 128, NB, NXT * 128)
            .transpose(1, 0, 2, 3)
        )
        f2p = np.pad(feat2[b], ((0, 0), (D, D), (D, D)))[
            :, half * HH : half * HH + HH + 8, :
        ]  # [256, 56, 104]
        m = {"f1": np.ascontiguousarray(f1td.astype(ml_dtypes.bfloat16))}
        for g, nm in enumerate(("f2a", "f2b", "f2c")):
            nr = GW[g] // 104
            sl = f2p[:, GR0[g] : GR0[g] + nr, :]  # [256, nr, 104]
            tg = sl.reshape(2, 128, GW[g]).transpose(1, 0, 2)
            m[nm] = np.ascontiguousarray(tg.astype(ml_dtypes.bfloat16))
        in_maps.append(m)

    res = run_bass_kernel_spmd(nc, in_maps, core_ids)

    out = np.empty((B, P * P, H, W), np.float32)
    for core in core_ids:
        b, half = divmod(core, 2)
        o = res.results[core]["o"]  # [3, 4, 32, 12, 192] bf16
        o = np.ascontiguousarray(o).astype(np.float32)
        # partition p = 32g + q; yi = 4g + q//8; xx = q%8
        # k(dy,dx) = 16*(q//8) + xx + 16*dy + dx
        o6 = o.reshape(NB, 4, 4, TX, NXT, RL)  # [blk, g, yj, xx, xt, k]
        tmp = np.empty((P * P, NB, 4, 4, NXT, TX), np.float32)
        for yj in range(4):
            for xx in range(TX):
                a = o6[:, :, yj, xx, :, 16 * yj + xx :]  # [3, 4, 12, >=137]
                sa = a.strides
                v = np.lib.stride_tricks.as_strided(
                    a, shape=(NB, 4, NXT, P, P),
                    strides=(sa[0], sa[1], sa[2], XW * sa[3], sa[3]),
                )
                # v[blk, g, xt, dy, dx]
                tmp[:, :, :, yj, :, xx] = v.transpose(3, 4, 0, 1, 2).reshape(
                    P * P, NB, 4, NXT
                )
        # y = blk*16 + g*4 + yj ; x = xt*8 + xx
        core_out = tmp.reshape(P * P, HH, W)
        out[b, :, half * HH : half * HH + HH, :] = core_out
    return out


# revision 20
# speedup vs baseline: 2.4149x; 1.1110x over previous
"""Trainium2 Bass kernel for nn_CostVolume (SpatialCorrelationSampler-style).

out[b, dy*9+dx, y, x] = sum_c feat1[b,c,y,x] * feat2_pad[b,c,y+dy,x+dx]
with feat2 zero-padded by 4 on H/W, dy/dx in [0,9), B=4, C=256, H=W=96.

Sharding (8 cores): core = (b, half) -- batch x H-half (48 rows each).

Device algorithm: 2D-tiled gram blocks. Per (y-block of 16, x-tile of 8):
  stationary = f1 tile [128c, 128=(16y x 8x)]  (128-col => FWL enabled)
  moving     = f2 window [128c, 384=(24r x 16x')], 1-2 MMs per C-half
  (row-split only at f2 group boundaries; col-split L/R tiles so compute
  starts after half a group arrives).
  psum[m=(yi,xx), n=r_rel*16+x_rel] accumulated over 2 C-halves.
  PSUM -> per-block SBUF stage [128, 12*384] bf16 (cast on copy).
  Output: per (blk, 32-partition group g) one rectangular DMA
  [[4608, 32], [384, 12], [1, 192]] at (partition 32g, col 64g): the
  union window covering all 81 displacements for those 4 yi groups
  (k = 16*(yi%4) + xx + 16*dy + dx); host strips via as_strided.
"""

import numpy as np
import ml_dtypes

import concourse.bacc as bacc
import concourse.mybir as mybir
from concourse.ap import AP
from concourse.tile import TileContext
from concourse.bass_utils import run_bass_kernel_spmd

B, C, H, W = 4, 256, 96, 96
D = 4
P = 2 * D + 1     # 9 displacements per axis
HH = H // 2       # 48 rows per core
NB = 3            # y-blocks of TY
TY, TX = 16, 8
NXT = W // TX     # 12 x-tiles
XW = TX + 8       # 16-col f2 window per tile
RL = 192          # per-32-partition-group read window (union over 4 yi)
STW = NXT * 384   # staged cols per block: 4608
GNR = (24, 16, 16)       # f2 group row counts: A=rows0-23, B=24-39, C=40-55
GR0 = (0, 24, 40)        # first f2 row of each group
HW2 = 56                 # half-width of f2 col split (L=[0:56], R=[48:104])

F32 = mybir.dt.float32
BF16 = mybir.dt.bfloat16

_CACHED = {}


def _build_nc():
    nc = bacc.Bacc()
    f1 = nc.declare_dram_parameter("f1", [128, 2, NB, NXT * 128], BF16, isOutput=False)
    f2d = {}
    for g in range(3):
        for s in "lr":
            nm = f"f2{'abc'[g]}{s}"
            f2d[(g, s)] = nc.declare_dram_parameter(
                nm, [128, 2, GNR[g] * HW2], BF16, isOutput=False
            )
    out = nc.declare_dram_parameter("o", [NB, 4, 32, NXT, RL], BF16, isOutput=True)

    with TileContext(nc) as tc:
        with (
            tc.tile_pool(name="w", bufs=1) as wp,
            tc.tile_pool(name="m", bufs=1) as mp,
            tc.tile_pool(name="st", bufs=3) as stp,
            tc.tile_pool(name="ps", bufs=6, space="PSUM") as psp,
            tc.tile_pool(name="wu", bufs=1, space="PSUM") as wup,
        ):
            # PE warmup: dummy matmuls on a memset tile while inputs load
            cst = wp.tile([128, 512], BF16, tag="cst", name="cst")
            nc.vector.memset(cst[:, :], 0)
            wups = wup.tile([128, 512], F32, tag="wups", name="wups")
            for _ in range(10):
                nc.tensor.matmul(
                    wups[:, 0:512], lhsT=cst[:, 0:128], rhs=cst[:, :],
                    start=True, stop=True,
                )

            f1t = [None] * NB
            f2t = {}

            def load_f1(blk):
                t = wp.tile([128, 2, NXT * 128], BF16, tag=f"f1b{blk}",
                            name=f"f1b{blk}")
                nc.sync.dma_start(out=t[:, :, :], in_=f1[:, :, blk, :])
                f1t[blk] = t

            def load_f2(g, s):
                t = mp.tile([128, 2, GNR[g] * HW2], BF16, tag=f"f2{g}{s}",
                            name=f"f2{g}{s}")
                nc.sync.dma_start(out=t[:, :, :], in_=f2d[(g, s)][:, :, :])
                f2t[(g, s)] = t

            load_f1(0)
            load_f2(0, "l")
            load_f2(0, "r")
            load_f1(1)
            load_f2(1, "l")
            load_f2(1, "r")
            load_f1(2)
            load_f2(2, "l")
            load_f2(2, "r")

            # per block: list of (group, row0_local_in_group, nrows, psum_col0)
            mm_plan = [
                [(0, 0, 24, 0)],
                [(0, 16, 8, 0), (1, 0, 16, 128)],
                [(1, 8, 8, 0), (2, 0, 16, 128)],
            ]

            out_engines = [nc.scalar, nc.gpsimd, nc.sync]
            ti = 0
            oi = 0
            for blk in range(NB):
                st = stp.tile([128, STW], BF16, tag="st", name="st")
                for xt in range(NXT):
                    side = "l" if xt < 6 else "r"
                    xoff = 8 * xt if xt < 6 else 8 * xt - 48
                    ps = psp.tile([128, 512], F32, tag="ps", name="ps")
                    for g, r0, nr, col0 in mm_plan[blk]:
                        mt = f2t[(g, side)]
                        gw = GNR[g] * HW2
                        for ch in range(2):
                            wt = f1t[blk]
                            lhsT = AP(
                                tensor=wt.tensor,
                                offset=wt.offset + ch * (NXT * 128) + xt * 128,
                                ap=[[2 * NXT * 128, 128], [1, 128]],
                            )
                            rhs = AP(
                                tensor=mt.tensor,
                                offset=mt.offset + ch * gw + r0 * HW2 + xoff,
                                ap=[[2 * gw, 128], [HW2, nr], [1, XW]],
                            )
                            nc.tensor.matmul(
                                ps[:, col0 : col0 + nr * XW],
                                lhsT=lhsT,
                                rhs=rhs,
                                start=(ch == 0),
                                stop=(ch == 1),
                            )
                    dst = st[:, xt * 384 : (xt + 1) * 384]
                    if ti % 2 == 0:
                        nc.vector.tensor_copy(dst, ps[:, 0:384])
                    else:
                        nc.scalar.copy(out=dst, in_=ps[:, 0:384])
                    ti += 1
                for g in range(4):
                    src = AP(
                        tensor=st.tensor,
                        offset=st.offset + 32 * g * STW + 64 * g,
                        ap=[[STW, 32], [384, NXT], [1, RL]],
                    )
                    out_engines[oi % 3].dma_start(out=out[blk, g], in_=src)
                    oi += 1
    nc.finalize()
    return nc


def kernel(feat1: np.ndarray, feat2: np.ndarray) -> np.ndarray:
    feat1 = np.ascontiguousarray(np.asarray(feat1, dtype=np.float32))
    feat2 = np.ascontiguousarray(np.asarray(feat2, dtype=np.float32))

    if "nc" not in _CACHED:
        _CACHED["nc"] = _build_nc()
    nc = _CACHED["nc"]

    core_ids = list(range(8))
    in_maps = []
    for core in core_ids:
        b, half = divmod(core, 2)
        f1h = feat1[b][:, half * HH : half * HH + HH, :]  # [256, 48, 96]
        # [c, blk, yi, xt, xx] -> [cl, ch, blk, (xt yi xx)]
        f1td = (
            f1h.reshape(256, NB, TY, NXT, TX)
            .transpose(0, 1, 3, 2, 4)
            .reshape(2, 128, NB, NXT * 128)
            .transpose(1, 0, 2, 3)
        )
        f2p = np.pad(feat2[b], ((0, 0), (D, D), (D, D)))[
            :, half * HH : half * HH + HH + 8, :
        ]  # [256, 56, 104]
        m = {"f1": np.ascontiguousarray(f1td.astype(ml_dtypes.bfloat16))}
        for g in range(3):
            for s, c0 in (("l", 0), ("r", 104 - HW2)):
                sl = f2p[:, GR0[g] : GR0[g] + GNR[g], c0 : c0 + HW2]
                tg = sl.reshape(2, 128, GNR[g] * HW2).transpose(1, 0, 2)
                m[f"f2{'abc'[g]}{s}"] = np.ascontiguousarray(
                    tg.astype(ml_dtypes.bfloat16)
                )
        in_maps.append(m)

    res = run_bass_kernel_spmd(nc, in_maps, core_ids)

    out = np.empty((B, P * P, H, W), np.float32)
    for core in core_ids:
        b, half = divmod(core, 2)
        o = res.results[core]["o"]  # [3, 4, 32, 12, 192] bf16
        o = np.ascontiguousarray(o).astype(np.float32)
        # partition p = 32g + q; yi = 4g + q//8; xx = q%8
        # k(dy,dx) = 16*(q//8) + xx + 16*dy + dx
        o6 = o.reshape(NB, 4, 4, TX, NXT, RL)  # [blk, g, yj, xx, xt, k]
        tmp = np.empty((P * P, NB, 4, 4, NXT, TX), np.float32)
        for yj in range(4):
            for xx in range(TX):
                a = o6[:, :, yj, xx, :, 16 * yj + xx :]  # [3, 4, 12, >=137]
                sa = a.strides
                v = np.lib.stride_tricks.as_strided(
                    a, shape=(NB, 4, NXT, P, P),
                    strides=(sa[0], sa[1], sa[2], XW * sa[3], sa[3]),
                )
                # v[blk, g, xt, dy, dx]
                tmp[:, :, :, yj, :, xx] = v.transpose(3, 4, 0, 1, 2).reshape(
                    P * P, NB, 4, NXT
                )
        # y = blk*16 + g*4 + yj ; x = xt*8 + xx
        core_out = tmp.reshape(P * P, HH, W)
        out[b, :, half * HH : half * HH + HH, :] = core_out
    return out


# revision 24
# speedup vs baseline: 2.6034x; 1.0780x over previous
"""Trainium2 Bass kernel for nn_CostVolume (SpatialCorrelationSampler-style).

out[b, dy*9+dx, y, x] = sum_c feat1[b,c,y,x] * feat2_pad[b,c,y+dy,x+dx]
with feat2 zero-padded by 4 on H/W, dy/dx in [0,9), B=4, C=256, H=W=96.

Sharding (8 cores): core = (b, half) -- batch x H-half (48 rows each).

Device algorithm: 2D-tiled gram blocks. Per (y-block of 16, x-tile of 8):
  stationary = f1 tile [128c, 128=(16y x 8x)]  (128-col => FWL enabled)
  moving     = f2 window [128c, 384=(24r x 16x')], 1-2 MMs per C-half
  (row-split only at f2 group boundaries; col-split L/R tiles so compute
  starts after half a group arrives).
  psum[m=(yi,xx), n=r_rel*16+x_rel] accumulated over 2 C-halves.
  PSUM -> per-block SBUF stage [128, 12*384] bf16 (cast on copy).
  Output: per (blk, 32-partition group g) one rectangular DMA
  [[4608, 32], [384, 12], [1, 192]] at (partition 32g, col 64g): the
  union window covering all 81 displacements for those 4 yi groups
  (k = 16*(yi%4) + xx + 16*dy + dx); host strips via as_strided.
"""

import numpy as np
import ml_dtypes

import concourse.bacc as bacc
import concourse.mybir as mybir
from concourse.ap import AP
from concourse.tile import TileContext
from concourse.bass_utils import run_bass_kernel_spmd

B, C, H, W = 4, 256, 96, 96
D = 4
P = 2 * D + 1     # 9 displacements per axis
HH = H // 2       # 48 rows per core
NB = 3            # y-blocks of TY
TY, TX = 16, 8
NXT = W // TX     # 12 x-tiles
XW = TX + 8       # 16-col f2 window per tile
RL = 192          # per-32-partition-group read window (union over 4 yi)
STW = NXT * 384   # staged cols per block: 4608
GNR = (24, 16, 16)       # f2 group row counts: A=rows0-23, B=24-39, C=40-55
GR0 = (0, 24, 40)        # first f2 row of each group
HW2 = 56                 # half-width of f2 col split (L=[0:56], R=[48:104])

F32 = mybir.dt.float32
BF16 = mybir.dt.bfloat16

_CACHED = {}


def _build_nc():
    nc = bacc.Bacc()
    f1 = nc.declare_dram_parameter("f1", [128, 2, NB, NXT * 128], BF16, isOutput=False)
    f2d = {}
    for g in range(3):
        for s in "lr":
            nm = f"f2{'abc'[g]}{s}"
            f2d[(g, s)] = nc.declare_dram_parameter(
                nm, [128, 2, GNR[g] * HW2], BF16, isOutput=False
            )
    out = nc.declare_dram_parameter("o", [NB, 4, 2, 32, 6, RL], BF16, isOutput=True)

    with TileContext(nc) as tc:
        with (
            tc.tile_pool(name="w", bufs=1) as wp,
            tc.tile_pool(name="m", bufs=1) as mp,
            tc.tile_pool(name="st", bufs=3) as stp,
            tc.tile_pool(name="ps", bufs=6, space="PSUM") as psp,
            tc.tile_pool(name="wu", bufs=1, space="PSUM") as wup,
        ):
            # PE warmup: dummy matmuls on a memset tile while inputs load
            cst = wp.tile([128, 512], BF16, tag="cst", name="cst")
            nc.vector.memset(cst[:, :], 0)
            wups = wup.tile([128, 512], F32, tag="wups", name="wups")
            for _ in range(18):
                nc.tensor.matmul(
                    wups[:, 0:512], lhsT=cst[:, 0:128], rhs=cst[:, :],
                    start=True, stop=True,
                )

            f1t = [None] * NB
            f2t = {}

            def load_f1(blk):
                t = wp.tile([128, 2, NXT * 128], BF16, tag=f"f1b{blk}",
                            name=f"f1b{blk}")
                nc.sync.dma_start(out=t[:, :, :], in_=f1[:, :, blk, :])
                f1t[blk] = t

            def load_f2(g, s):
                t = mp.tile([128, 2, GNR[g] * HW2], BF16, tag=f"f2{g}{s}",
                            name=f"f2{g}{s}")
                nc.sync.dma_start(out=t[:, :, :], in_=f2d[(g, s)][:, :, :])
                f2t[(g, s)] = t

            load_f1(0)
            load_f2(0, "l")
            load_f2(0, "r")
            load_f1(1)
            load_f2(1, "l")
            load_f2(1, "r")
            load_f1(2)
            load_f2(2, "l")
            load_f2(2, "r")

            # per block: list of (group, row0_local_in_group, nrows, psum_col0)
            mm_plan = [
                [(0, 0, 24, 0)],
                [(0, 16, 8, 0), (1, 0, 16, 128)],
                [(1, 8, 8, 0), (2, 0, 16, 128)],
            ]

            out_engines = [nc.scalar, nc.gpsimd, nc.sync]
            ti = 0
            oi = 0
            for blk in range(NB):
                st = stp.tile([128, STW], BF16, tag="st", name="st")
                for xt in range(NXT):
                    side = "l" if xt < 6 else "r"
                    xoff = 8 * xt if xt < 6 else 8 * xt - 48
                    ps = psp.tile([128, 512], F32, tag="ps", name="ps")
                    for g, r0, nr, col0 in mm_plan[blk]:
                        mt = f2t[(g, side)]
                        gw = GNR[g] * HW2
                        for ch in range(2):
                            wt = f1t[blk]
                            lhsT = AP(
                                tensor=wt.tensor,
                                offset=wt.offset + ch * (NXT * 128) + xt * 128,
                                ap=[[2 * NXT * 128, 128], [1, 128]],
                            )
                            rhs = AP(
                                tensor=mt.tensor,
                                offset=mt.offset + ch * gw + r0 * HW2 + xoff,
                                ap=[[2 * gw, 128], [HW2, nr], [1, XW]],
                            )
                            nc.tensor.matmul(
                                ps[:, col0 : col0 + nr * XW],
                                lhsT=lhsT,
                                rhs=rhs,
                                start=(ch == 0),
                                stop=(ch == 1),
                            )
                    dst = st[:, xt * 384 : (xt + 1) * 384]
                    if ti % 2 == 0:
                        nc.vector.tensor_copy(dst, ps[:, 0:384])
                    else:
                        nc.scalar.copy(out=dst, in_=ps[:, 0:384])
                    ti += 1
                for g in range(4):
                    for h in range(2):
                        src = AP(
                            tensor=st.tensor,
                            offset=st.offset + 32 * g * STW + 64 * g
                            + h * 6 * 384,
                            ap=[[STW, 32], [384, 6], [1, RL]],
                        )
                        out_engines[oi % 3].dma_start(out=out[blk, g, h], in_=src)
                        oi += 1
    nc.finalize()
    return nc


def kernel(feat1: np.ndarray, feat2: np.ndarray) -> np.ndarray:
    feat1 = np.ascontiguousarray(np.asarray(feat1, dtype=np.float32))
    feat2 = np.ascontiguousarray(np.asarray(feat2, dtype=np.float32))

    if "nc" not in _CACHED:
        _CACHED["nc"] = _build_nc()
    nc = _CACHED["nc"]

    core_ids = list(range(8))
    in_maps = []
    for core in core_ids:
        b, half = divmod(core, 2)
        f1h = feat1[b][:, half * HH : half * HH + HH, :]  # [256, 48, 96]
        # [c, blk, yi, xt, xx] -> [cl, ch, blk, (xt yi xx)]
        f1td = (
            f1h.reshape(256, NB, TY, NXT, TX)
            .transpose(0, 1, 3, 2, 4)
            .reshape(2, 128, NB, NXT * 128)
            .transpose(1, 0, 2, 3)
        )
        f2p = np.pad(feat2[b], ((0, 0), (D, D), (D, D)))[
            :, half * HH : half * HH + HH + 8, :
        ]  # [256, 56, 104]
        m = {"f1": np.ascontiguousarray(f1td.astype(ml_dtypes.bfloat16))}
        for g in range(3):
            for s, c0 in (("l", 0), ("r", 104 - HW2)):
                sl = f2p[:, GR0[g] : GR0[g] + GNR[g], c0 : c0 + HW2]
                tg = sl.reshape(2, 128, GNR[g] * HW2).transpose(1, 0, 2)
                m[f"f2{'abc'[g]}{s}"] = np.ascontiguousarray(
                    tg.astype(ml_dtypes.bfloat16)
                )
        in_maps.append(m)

    res = run_bass_kernel_spmd(nc, in_maps, core_ids)

    out = np.empty((B, P * P, H, W), np.float32)
    for core in core_ids:
        b, half = divmod(core, 2)
        o = res.results[core]["o"]  # [3, 4, 2, 32, 6, 192] bf16
        o = np.ascontiguousarray(o).astype(np.float32)
        # partition p = 32g + q; yi = 4g + q//8; xx = q%8; xt = 6h + xtl
        # k(dy,dx) = 16*(q//8) + xx + 16*dy + dx
        o7 = o.reshape(NB, 4, 2, 4, TX, 6, RL)  # [blk, g, h, yj, xx, xtl, k]
        o6 = o7.transpose(0, 1, 3, 4, 2, 5, 6)  # [blk, g, yj, xx, h, xtl, k]
        tmp = np.empty((P * P, NB, 4, 4, NXT, TX), np.float32)
        for yj in range(4):
            for xx in range(TX):
                a = o6[:, :, yj, xx, :, :, 16 * yj + xx :]  # [3,4,2,6,>=137]
                sa = a.strides
                v = np.lib.stride_tricks.as_strided(
                    a, shape=(NB, 4, 2, 6, P, P),
                    strides=(sa[0], sa[1], sa[2], sa[3], XW * sa[4], sa[4]),
                )
                # v[blk, g, h, xtl, dy, dx]
                tmp[:, :, :, yj, :, xx] = v.transpose(4, 5, 0, 1, 2, 3).reshape(
                    P * P, NB, 4, NXT
                )
        # y = blk*16 + g*4 + yj ; x = xt*8 + xx
        core_out = tmp.reshape(P * P, HH, W)
        out[b, :, half * HH : half * HH + HH, :] = core_out
    return out


# revision 28
# speedup vs baseline: 2.6443x; 1.0157x over previous
"""Trainium2 Bass kernel for nn_CostVolume (SpatialCorrelationSampler-style).

out[b, dy*9+dx, y, x] = sum_c feat1[b,c,y,x] * feat2_pad[b,c,y+dy,x+dx]
with feat2 zero-padded by 4 on H/W, dy/dx in [0,9), B=4, C=256, H=W=96.

Sharding (8 cores): core = (b, half) -- batch x H-half (48 rows each).

Device algorithm: 2D-tiled gram blocks. Per (y-block of 16, x-tile of 8):
  stationary = f1 tile [128c, 128=(16y x 8x)]  (128-col => FWL enabled)
  moving     = f2 window [128c, 384=(24r x 16x')], 1-2 MMs per C-half
  (row-split only at f2 group boundaries; col-split L/R tiles so compute
  starts after half a group arrives).
  psum[m=(yi,xx), n=r_rel*16+x_rel] accumulated over 2 C-halves.
  PSUM -> per-block SBUF stage [128, 12*384] bf16 (cast on copy).
  Output: per (blk, 32-partition group g) one rectangular DMA
  [[4608, 32], [384, 12], [1, 192]] at (partition 32g, col 64g): the
  union window covering all 81 displacements for those 4 yi groups
  (k = 16*(yi%4) + xx + 16*dy + dx); host strips via as_strided.
"""

import numpy as np
import ml_dtypes

import concourse.bacc as bacc
import concourse.mybir as mybir
from concourse.ap import AP
from concourse.tile import TileContext
from concourse.bass_utils import run_bass_kernel_spmd

B, C, H, W = 4, 256, 96, 96
D = 4
P = 2 * D + 1     # 9 displacements per axis
HH = H // 2       # 48 rows per core
NB = 3            # y-blocks of TY
TY, TX = 16, 8
NXT = W // TX     # 12 x-tiles
XW = TX + 8       # 16-col f2 window per tile
RL = 192          # per-32-partition-group read window (union over 4 yi)
STW = NXT * 384   # staged cols per block: 4608
GNR = (24, 16, 16)       # f2 group row counts: A=rows0-23, B=24-39, C=40-55
GR0 = (0, 24, 40)        # first f2 row of each group
HW2 = 56                 # half-width of f2 col split (L=[0:56], R=[48:104])

F32 = mybir.dt.float32
BF16 = mybir.dt.bfloat16

_CACHED = {}


def _build_nc():
    nc = bacc.Bacc()
    f1 = nc.declare_dram_parameter("f1", [128, 2, NB, NXT * 128], BF16, isOutput=False)
    f2d = {}
    for g in range(3):
        for s in "lr":
            nm = f"f2{'abc'[g]}{s}"
            f2d[(g, s)] = nc.declare_dram_parameter(
                nm, [128, 2, GNR[g] * HW2], BF16, isOutput=False
            )
    out = nc.declare_dram_parameter("o", [NB, 4, 2, 32, 6, RL], BF16, isOutput=True)

    with TileContext(nc) as tc:
        with (
            tc.tile_pool(name="w", bufs=1) as wp,
            tc.tile_pool(name="m", bufs=1) as mp,
            tc.tile_pool(name="st", bufs=3) as stp,
            tc.tile_pool(name="ps", bufs=6, space="PSUM") as psp,
            tc.tile_pool(name="wu", bufs=1, space="PSUM") as wup,
        ):
            # PE warmup: dummy matmuls on a memset tile while inputs load
            cst = wp.tile([128, 512], BF16, tag="cst", name="cst")
            nc.vector.memset(cst[:, :], 0)
            wups = wup.tile([128, 512], F32, tag="wups", name="wups")
            for _ in range(11):
                nc.tensor.matmul(
                    wups[:, 0:384], lhsT=cst[:, 0:128], rhs=cst[:, 0:384],
                    start=True, stop=True,
                )

            f1t = [None] * NB
            f2t = {}

            def load_f1(blk):
                t = wp.tile([128, 2, NXT * 128], BF16, tag=f"f1b{blk}",
                            name=f"f1b{blk}")
                nc.sync.dma_start(out=t[:, :, :], in_=f1[:, :, blk, :])
                f1t[blk] = t

            def load_f2(g, s):
                t = mp.tile([128, 2, GNR[g] * HW2], BF16, tag=f"f2{g}{s}",
                            name=f"f2{g}{s}")
                nc.sync.dma_start(out=t[:, :, :], in_=f2d[(g, s)][:, :, :])
                f2t[(g, s)] = t

            load_f1(0)
            load_f2(0, "l")
            load_f2(0, "r")
            load_f1(1)
            load_f2(1, "l")
            load_f2(1, "r")
            load_f1(2)
            load_f2(2, "l")
            load_f2(2, "r")

            # per block: list of (group, row0_local_in_group, nrows, psum_col0)
            mm_plan = [
                [(0, 0, 24, 0)],
                [(0, 16, 8, 0), (1, 0, 16, 128)],
                [(1, 8, 8, 0), (2, 0, 16, 128)],
            ]

            out_engines = [nc.gpsimd, nc.sync]
            ti = 0
            oi = 0
            for blk in range(NB):
                st = stp.tile([128, STW], BF16, tag="st", name="st")
                for xt in range(NXT):
                    side = "l" if xt < 6 else "r"
                    xoff = 8 * xt if xt < 6 else 8 * xt - 48
                    ps = psp.tile([128, 512], F32, tag="ps", name="ps")
                    for g, r0, nr, col0 in mm_plan[blk]:
                        mt = f2t[(g, side)]
                        gw = GNR[g] * HW2
                        for ch in range(2):
                            wt = f1t[blk]
                            lhsT = AP(
                                tensor=wt.tensor,
                                offset=wt.offset + ch * (NXT * 128) + xt * 128,
                                ap=[[2 * NXT * 128, 128], [1, 128]],
                            )
                            rhs = AP(
                                tensor=mt.tensor,
                                offset=mt.offset + ch * gw + r0 * HW2 + xoff,
                                ap=[[2 * gw, 128], [HW2, nr], [1, XW]],
                            )
                            nc.tensor.matmul(
                                ps[:, col0 : col0 + nr * XW],
                                lhsT=lhsT,
                                rhs=rhs,
                                start=(ch == 0),
                                stop=(ch == 1),
                            )
                    dst = st[:, xt * 384 : (xt + 1) * 384]
                    nc.vector.tensor_copy(dst[:, 0:192], ps[:, 0:192])
                    nc.scalar.copy(out=dst[:, 192:384], in_=ps[:, 192:384])
                    ti += 1
                for g in range(4):
                    for h in range(2):
                        src = AP(
                            tensor=st.tensor,
                            offset=st.offset + 32 * g * STW + 64 * g
                            + h * 6 * 384,
                            ap=[[STW, 32], [384, 6], [1, RL]],
                        )
                        out_engines[oi % 2].dma_start(out=out[blk, g, h], in_=src)
                        oi += 1
    nc.finalize()
    return nc


def kernel(feat1: np.ndarray, feat2: np.ndarray) -> np.ndarray:
    feat1 = np.ascontiguousarray(np.asarray(feat1, dtype=np.float32))
    feat2 = np.ascontiguousarray(np.asarray(feat2, dtype=np.float32))

    if "nc" not in _CACHED:
        _CACHED["nc"] = _build_nc()
    nc = _CACHED["nc"]

    core_ids = list(range(8))
    in_maps = []
    for core in core_ids:
        b, half = divmod(core, 2)
        f1h = feat1[b][:, half * HH : half * HH + HH, :]  # [256, 48, 96]
        # [c, blk, yi, xt, xx] -> [cl, ch, blk, (xt yi xx)]
        f1td = (
            f1h.reshape(256, NB, TY, NXT, TX)
            .transpose(0, 1, 3, 2, 4)
            .reshape(2, 128, NB, NXT * 128)
            .transpose(1, 0, 2, 3)
        )
        f2p = np.pad(feat2[b], ((0, 0), (D, D), (D, D)))[
            :, half * HH : half * HH + HH + 8, :
        ]  # [256, 56, 104]
        m = {"f1": np.ascontiguousarray(f1td.astype(ml_dtypes.bfloat16))}
        for g in range(3):
            for s, c0 in (("l", 0), ("r", 104 - HW2)):
                sl = f2p[:, GR0[g] : GR0[g] + GNR[g], c0 : c0 + HW2]
                tg = sl.reshape(2, 128, GNR[g] * HW2).transpose(1, 0, 2)
                m[f"f2{'abc'[g]}{s}"] = np.ascontiguousarray(
                    tg.astype(ml_dtypes.bfloat16)
                )
        in_maps.append(m)

    res = run_bass_kernel_spmd(nc, in_maps, core_ids)

    out = np.empty((B, P * P, H, W), np.float32)
    for core in core_ids:
        b, half = divmod(core, 2)
        o = res.results[core]["o"]  # [3, 4, 2, 32, 6, 192] bf16
        o = np.ascontiguousarray(o).astype(np.float32)
        # partition p = 32g + q; yi = 4g + q//8; xx = q%8; xt = 6h + xtl
        # k(dy,dx) = 16*(q//8) + xx + 16*dy + dx
        o7 = o.reshape(NB, 4, 2, 4, TX, 6, RL)  # [blk, g, h, yj, xx, xtl, k]
        o6 = o7.transpose(0, 1, 3, 4, 2, 5, 6)  # [blk, g, yj, xx, h, xtl, k]
        tmp = np.empty((P * P, NB, 4, 4, NXT, TX), np.float32)
        for yj in range(4):
            for xx in range(TX):
                a = o6[:, :, yj, xx, :, :, 16 * yj + xx :]  # [3,4,2,6,>=137]
                sa = a.strides
                v = np.lib.stride_tricks.as_strided(
                    a, shape=(NB, 4, 2, 6, P, P),
                    strides=(sa[0], sa[1], sa[2], sa[3], XW * sa[4], sa[4]),
                )
                # v[blk, g, h, xtl, dy, dx]
                tmp[:, :, :, yj, :, xx] = v.transpose(4, 5, 0, 1, 2, 3).reshape(
                    P * P, NB, 4, NXT
                )
        # y = blk*16 + g*4 + yj ; x = xt*8 + xx
        core_out = tmp.reshape(P * P, HH, W)
        out[b, :, half * HH : half * HH + HH, :] = core_out
    return out
